# revision 1
# baseline (speedup 1.0000x reference)
"""Trainium2 Bass kernel for nn_AttentionEncoderModel (8 NeuronCores).

Strategy: pure data-parallel over batch (B=8 -> 1 element/core), all params
replicated, bf16 TensorE matmuls with fp32 accumulation. Activations are kept
in "transposed" layout [features(partitions), rows(free)] so every matmul is
lhsT = weight tile [K=128, M<=128], rhs = activation tile [K=128, N=512].
The only cross-core communication is an 8-byte AllReduce for the final global
standardization.
"""

import numpy as np
import ml_dtypes

import concourse.bass as bass
import concourse.mybir as mybir
from concourse import bacc
from concourse.tile import TileContext
from concourse.bass_utils import run_bass_kernel_spmd

AF = mybir.ActivationFunctionType
OP = mybir.AluOpType
BF = mybir.dt.bfloat16
F32 = mybir.dt.float32

P = 128
ROWS = 512
B, S, D = 8, 512, 256
H, DH = 8, 32
NB = 8
COMP = 128
LN_EPS = 1e-5
SCALE = 1.0 / np.sqrt(DH)
NEG = -1e9

# fc layer dims (K_in, M_out) and activation
FC_DIMS = [(4096, 4096), (4096, 2048), (2048, 1024), (1024, 512), (512, 256)]


def build_nc(n_cores=8):
    nc = bacc.Bacc("TRN2", target_bir_lowering=False, debug=False,
                   num_devices=n_cores)
    NTOT = float(n_cores * ROWS * COMP)

    # ---------------- DRAM parameters ----------------
    xT = nc.declare_dram_parameter("xT", [32, P, ROWS], BF, False)
    fc_w, fc_b = [], []
    for i, (kin, mout) in enumerate(FC_DIMS + [(256, 256)]):  # + pre layer
        mt, kt = mout // P, kin // P
        fc_w.append(nc.declare_dram_parameter(f"w{i}", [mt, P, kt * P], BF, False))
        fc_b.append(nc.declare_dram_parameter(f"b{i}", [P, mt], F32, False))
    posT_d = nc.declare_dram_parameter("posT", [2, P, ROWS], F32, False)
    encqk_d = nc.declare_dram_parameter("encqk", [NB, P, 1024], BF, False)
    encv_d = nc.declare_dram_parameter("encv", [NB, P, 512], BF, False)
    encbqk_d = nc.declare_dram_parameter("encbqk", [NB, P, 4], F32, False)
    bvb_d = nc.declare_dram_parameter("bvb", [NB, P, 256], F32, False)
    lngb_d = nc.declare_dram_parameter("lngb", [NB, 2, 512], BF, False)
    rw1_d = nc.declare_dram_parameter("rw1", [NB, P, 2048], BF, False)
    rb1_d = nc.declare_dram_parameter("rb1", [NB, P, 8], F32, False)
    rw2_d = nc.declare_dram_parameter("rw2", [NB, P, 2048], BF, False)
    rb2_d = nc.declare_dram_parameter("rb2", [NB, P, 2], F32, False)
    outw_d = nc.declare_dram_parameter("outw", [P, 256], BF, False)
    outb_d = nc.declare_dram_parameter("outb", [P, 1], F32, False)
    tri_d = nc.declare_dram_parameter("tri", [P, P], F32, False)
    m256_d = nc.declare_dram_parameter("m256", [P, 1], BF, False)
    onesP_d = nc.declare_dram_parameter("onesP", [P, 1], BF, False)
    lnones_d = nc.declare_dram_parameter("lnones", [1, ROWS], BF, False)

    out_d = nc.declare_dram_parameter("out", [P, ROWS], F32, True)

    with TileContext(nc) as tc:
        with (
            tc.tile_pool(name="const", bufs=1) as cpool,
            tc.tile_pool(name="stream", bufs=1) as spool,
            tc.tile_pool(name="wpool", bufs=3) as wpool,
            tc.tile_pool(name="dram", bufs=1, space="DRAM") as dpool,
        ):
            # constants
            tri_sb = cpool.tile([P, P], F32, name="tri_sb")
            nc.sync.dma_start(tri_sb[:], tri_d[:])
            m256_sb = cpool.tile([P, 1], BF, name="m256_sb")
            nc.sync.dma_start(m256_sb[:], m256_d[:])
            onesP_sb = cpool.tile([P, 1], BF, name="onesP_sb")
            nc.sync.dma_start(onesP_sb[:], onesP_d[:])
            lnones_sb = cpool.tile([1, ROWS], BF, name="lnones_sb")
            nc.sync.dma_start(lnones_sb[:], lnones_d[:])

            cconst = cpool.tile([P, 2], F32, name="cconst")
            nc.vector.memset(cconst[:, 0:1], 0.0)
            nc.vector.memset(cconst[:, 1:2], LN_EPS)
            nc.const_aps.aps[(F32, 0.0)] = cconst[:, 0:1]
            nc.const_aps.aps[(F32, LN_EPS)] = cconst[:, 1:2]

            # residual stream x^T [256, 512] f32 as 2 tiles
            xs = []
            for m in range(2):
                t = spool.tile([P, ROWS], F32, name=f"xs_{m}")
                xs.append(t)

            # ---------------- MLP front ----------------
            with tc.tile_pool(name="acts", bufs=1) as apool, \
                 tc.tile_pool(name="mlp_ps", bufs=3, space="PSUM") as mpp:
                cur = []
                for k in range(32):
                    t = apool.tile([P, ROWS], BF, name=f"x0_{k}")
                    nc.sync.dma_start(t[:], xT[k])
                    cur.append(t)

                for i, (kin, mout) in enumerate(FC_DIMS):
                    mt, kt = mout // P, kin // P
                    bias_sb = apool.tile([P, mt], F32, name=f"bias{i}")
                    nc.sync.dma_start(bias_sb[:], fc_b[i][:])
                    act = AF.Tanh if i == 4 else AF.Relu
                    nxt = []
                    for m in range(mt):
                        w_sb = wpool.tile([P, kt * P], BF, tag="wmlp",
                                          name=f"w{i}_{m}")
                        nc.sync.dma_start(w_sb[:], fc_w[i][m])
                        ps = mpp.tile([P, ROWS], F32, tag="mlp", name=f"ps{i}_{m}")
                        for k in range(kt):
                            nc.tensor.matmul(ps[:], w_sb[:, k * P:(k + 1) * P],
                                             cur[k][:], start=(k == 0),
                                             stop=(k == kt - 1))
                        o = apool.tile([P, ROWS], BF, name=f"a{i}_{m}")
                        nc.scalar.activation(o[:], ps[:], act,
                                             bias=bias_sb[:, m:m + 1])
                        nxt.append(o)
                    cur = nxt

                # pre layer -> f32 stream + positional
                posT_sb = apool.tile([P, 2 * ROWS], F32, name="posT_sb")
                posT_v = posT_sb.rearrange("p (m r) -> p m r", m=2)
                nc.sync.dma_start(posT_v[:], posT_d.rearrange("m p r -> p m r"))
                bias_sb = apool.tile([P, 2], F32, name="bias5")
                nc.sync.dma_start(bias_sb[:], fc_b[5][:])
                for m in range(2):
                    w_sb = wpool.tile([P, 2 * P], BF, tag="wmlp", name=f"w5_{m}")
                    nc.sync.dma_start(w_sb[:], fc_w[5][m])
                    ps = mpp.tile([P, ROWS], F32, tag="mlp", name=f"ps5_{m}")
                    for k in range(2):
                        nc.tensor.matmul(ps[:], w_sb[:, k * P:(k + 1) * P],
                                         cur[k][:], start=(k == 0), stop=(k == 1))
                    nc.vector.scalar_tensor_tensor(
                        xs[m][:], ps[:], bias_sb[:, m:m + 1], posT_v[:, m, :],
                        op0=OP.add, op1=OP.add)

            # ---------------- transformer blocks ----------------
            def layernorm(l, which, bpool, xn_out_bf, replace_stream):
                """LN over features (partition dim) of xs; writes bf16 tiles
                xn_out_bf[m]; if replace_stream, also overwrites xs[m] (f32).
                gb_sb row layout: [1, 512] = [gamma(256) | beta(256)]."""
                gb_sb = bpool.tile([1, 512], BF, tag="lngb", name=f"gb_{l}_{which}")
                nc.sync.dma_start(gb_sb[:], lngb_d[l, which])
                with tc.tile_pool(name=f"lnps_{l}_{which}", bufs=1,
                                  space="PSUM") as lpp:
                    mu_ps = lpp.tile([1, ROWS], F32, name=f"mu_{l}_{which}")
                    sq_ps = lpp.tile([1, ROWS], F32, name=f"sq_{l}_{which}")
                    for m in range(2):
                        xbf = bpool.tile([P, ROWS], BF, tag="ln_xbf",
                                         name=f"lnxbf_{l}_{which}_{m}")
                        nc.vector.tensor_copy(xbf[:], xs[m][:])
                        sqbf = bpool.tile([P, ROWS], BF, tag="ln_sqbf",
                                          name=f"lnsq_{l}_{which}_{m}")
                        nc.scalar.activation(sqbf[:], xs[m][:], AF.Square)
                        nc.tensor.matmul(mu_ps[:], m256_sb[:], xbf[:],
                                         start=(m == 0), stop=(m == 1))
                        nc.tensor.matmul(sq_ps[:], m256_sb[:], sqbf[:],
                                         start=(m == 0), stop=(m == 1))
                    # row math
                    t1 = bpool.tile([1, ROWS], F32, tag="ln_t1",
                                    name=f"lnt1_{l}_{which}")
                    nc.scalar.activation(t1[:], mu_ps[:], AF.Square)
                    var = bpool.tile([1, ROWS], F32, tag="ln_var",
                                     name=f"lnvar_{l}_{which}")
                    nc.vector.tensor_tensor(var[:], sq_ps[:], t1[:],
                                            op=OP.subtract)
                    lnv = bpool.tile([1, ROWS], F32, tag="ln_lnv",
                                     name=f"lnlnv_{l}_{which}")
                    nc.scalar.activation(lnv[:], var[:], AF.Ln, bias=LN_EPS)
                    rstd = bpool.tile([1, ROWS], F32, tag="ln_rstd",
                                      name=f"lnrstd_{l}_{which}")
                    nc.scalar.activation(rstd[:], lnv[:], AF.Exp, scale=-0.5)
                    rstd_bf = bpool.tile([1, ROWS], BF, tag="ln_rstdbf",
                                         name=f"lnrstdbf_{l}_{which}")
                    nc.vector.tensor_copy(rstd_bf[:], rstd[:])
                    nmr_bf = bpool.tile([1, ROWS], BF, tag="ln_nmr",
                                        name=f"lnnmr_{l}_{which}")
                    nc.vector.scalar_tensor_tensor(
                        nmr_bf[:], mu_ps[:], -1.0, rstd[:],
                        op0=OP.mult, op1=OP.mult)
                    for m in range(2):
                        a_ps = lpp.tile([P, ROWS], F32, tag="ln_ab", bufs=2,
                                        name=f"lnA_{l}_{which}_{m}")
                        nc.tensor.matmul(a_ps[:], gb_sb[0:1, m * P:(m + 1) * P],
                                         rstd_bf[:], start=True, stop=True)
                        b_ps = lpp.tile([P, ROWS], F32, tag="ln_ab", bufs=2,
                                        name=f"lnB_{l}_{which}_{m}")
                        nc.tensor.matmul(b_ps[:], gb_sb[0:1, m * P:(m + 1) * P],
                                         nmr_bf[:], start=True, stop=False)
                        nc.tensor.matmul(b_ps[:],
                                         gb_sb[0:1, 256 + m * P:256 + (m + 1) * P],
                                         lnones_sb[:], start=False, stop=True)
                        tmp = bpool.tile([P, ROWS], F32, tag="ln_tmp",
                                         name=f"lntmp_{l}_{which}_{m}")
                        nc.vector.tensor_tensor(tmp[:], xs[m][:], a_ps[:],
                                                op=OP.mult)
                        if replace_stream:
                            nc.vector.tensor_tensor(xs[m][:], tmp[:], b_ps[:],
                                                    op=OP.add)
                            nc.vector.tensor_copy(xn_out_bf[m][:], xs[m][:])
                        else:
                            nc.vector.tensor_tensor(xn_out_bf[m][:], tmp[:],
                                                    b_ps[:], op=OP.add)

            for l in range(NB):
                with tc.tile_pool(name=f"blk_{l}", bufs=1) as bpool:
                    # ---- ln1 -> xn1 (bf16 only)
                    xn1 = [bpool.tile([P, ROWS], BF, tag=f"xn1_{m}",
                                      name=f"xn1_{l}_{m}") for m in range(2)]
                    layernorm(l, 0, bpool, xn1, replace_stream=False)

                    # ---- qkv weights for this block
                    eqk_sb = bpool.tile([P, 1024], BF, tag="eqk",
                                        name=f"eqk_{l}")
                    nc.sync.dma_start(eqk_sb[:], encqk_d[l])
                    ev_sb = bpool.tile([P, 512], BF, tag="ev", name=f"ev_{l}")
                    nc.sync.dma_start(ev_sb[:], encv_d[l])
                    ebqk_sb = bpool.tile([P, 4], F32, tag="ebqk",
                                         name=f"ebqk_{l}")
                    nc.sync.dma_start(ebqk_sb[:], encbqk_d[l])
                    bvb_sb = bpool.tile([P, 256], F32, tag="bvb",
                                        name=f"bvb_{l}")
                    nc.sync.dma_start(bvb_sb[:], bvb_d[l])

                    eqk_v = eqk_sb.rearrange("p (m k c) -> p m k c", m=4, k=2)
                    qk_bf = []
                    with tc.tile_pool(name=f"qkps_{l}", bufs=2,
                                      space="PSUM") as qpp:
                        for mt in range(4):
                            ps = qpp.tile([P, ROWS], F32, tag="qk",
                                          name=f"qkps_{l}_{mt}")
                            for k in range(2):
                                nc.tensor.matmul(ps[:], eqk_v[:, mt, k, :],
                                                 xn1[k][:], start=(k == 0),
                                                 stop=(k == 1))
                            o = bpool.tile([P, ROWS], BF, tag=f"qk_{mt}",
                                           name=f"qkbf_{l}_{mt}")
                            nc.scalar.activation(o[:], ps[:], AF.Identity,
                                                 bias=ebqk_sb[:, mt:mt + 1])
                            qk_bf.append(o)
                        # V (natural layout) + aug with ones column
                        ev_v = ev_sb.rearrange("p (k c) -> p k c", k=2)
                        v_aug = []
                        for rt in range(4):
                            ps = qpp.tile([P, 256], F32, tag="v",
                                          name=f"vps_{l}_{rt}")
                            for k in range(2):
                                nc.tensor.matmul(
                                    ps[:], xn1[k][:, rt * P:(rt + 1) * P],
                                    ev_v[:, k, :], start=(k == 0), stop=(k == 1))
                            va = bpool.tile([P, 264], BF, tag=f"vaug_{rt}",
                                            name=f"vaug_{l}_{rt}")
                            va_v = va.rearrange("p (h c) -> p h c", c=33)
                            nc.vector.scalar_tensor_tensor(
                                va_v[:, :, 0:32],
                                ps.rearrange("p (h c) -> p h c", c=32),
                                1.0,
                                bvb_sb.rearrange("p (h c) -> p h c", c=32),
                                op0=OP.mult, op1=OP.add)
                            nc.vector.memset(va_v[:, :, 32:33], 1.0)
                            v_aug.append(va)

                    # ---- attention per head-group (heads 4g..4g+3 -> x tile g)
                    for g in range(2):
                        with tc.tile_pool(name=f"att_{l}_{g}", bufs=1,
                                          space="PSUM") as app:
                            expS = {}
                            for t in range(4):
                                for hh in range(4):
                                    s_ps = app.tile([P, ROWS], F32, tag="s",
                                                    bufs=4,
                                                    name=f"sps_{l}_{g}_{t}_{hh}")
                                    lhsT = qk_bf[2 + g][32 * hh:32 * hh + 32,
                                                        t * P:(t + 1) * P]
                                    rhs = qk_bf[g][32 * hh:32 * hh + 32, :]
                                    nc.tensor.matmul(s_ps[:], lhsT, rhs,
                                                     start=True, stop=True,
                                                     tile_position=(32 * hh, 0))
                                    nc.vector.tensor_tensor(
                                        s_ps[:, t * P:(t + 1) * P],
                                        s_ps[:, t * P:(t + 1) * P],
                                        tri_sb[:], op=OP.add)
                                    e = bpool.tile([P, ROWS], BF,
                                                   tag=f"expS_{hh}_{t}",
                                                   name=f"expS_{l}_{g}_{hh}_{t}")
                                    if t > 0:
                                        nc.vector.memset(e[:, 0:t * P], 0.0)
                                    nc.scalar.activation(
                                        e[:, t * P:], s_ps[:, t * P:], AF.Exp,
                                        scale=SCALE)
                                    expS[(hh, t)] = e
                            dbf = [bpool.tile([1, ROWS], BF, tag=f"dbf_{hh}",
                                              name=f"dbf_{l}_{g}_{hh}")
                                   for hh in range(4)]
                            pv_tiles = []
                            for pi in range(2):
                                hh0, hh1 = 2 * pi, 2 * pi + 1
                                pv = app.tile([P, ROWS], F32, tag="pv", bufs=2,
                                              name=f"pv_{l}_{g}_{pi}")
                                gA, gB = 4 * g + hh0, 4 * g + hh1
                                for t in range(4):
                                    nc.tensor.matmul(
                                        pv[0:33, :],
                                        v_aug[t][:, 33 * gA:33 * gA + 33],
                                        expS[(hh0, t)][:],
                                        start=(t == 0), stop=(t == 3),
                                        tile_position=(0, 0),
                                        skip_group_check=True)
                                    nc.tensor.matmul(
                                        pv[64:97, :],
                                        v_aug[t][:, 33 * gB:33 * gB + 33],
                                        expS[(hh1, t)][:],
                                        start=(t == 0), stop=(t == 3),
                                        tile_position=(0, 64),
                                        skip_group_check=True)
                                for hh, prow in ((hh0, 32), (hh1, 96)):
                                    dr = bpool.tile([1, ROWS], F32, tag="drec",
                                                    name=f"dr_{l}_{g}_{hh}")
                                    nc.vector.reciprocal(dr[:],
                                                         pv[prow:prow + 1, :])
                                    nc.vector.tensor_copy(dbf[hh][:], dr[:])
                                pv_tiles.append(pv)
                            r_ps = app.tile([P, ROWS], F32, tag="r", bufs=1,
                                            name=f"r_{l}_{g}")
                            for q in range(4):
                                nc.tensor.matmul(r_ps[32 * q:32 * q + 32, :],
                                                 lnones_sb[0:1, 0:32],
                                                 dbf[q][:],
                                                 start=True, stop=True,
                                                 tile_position=(0, 32 * q))
                            at_sb = bpool.tile([P, ROWS], F32, tag="at_sb",
                                               name=f"atsb_{l}_{g}")
                            for q in range(4):
                                off = 64 * (q % 2)
                                nc.scalar.activation(
                                    at_sb[32 * q:32 * q + 32, :],
                                    pv_tiles[q // 2][off:off + 32, :], AF.Copy)
                                nc.vector.tensor_tensor(
                                    at_sb[32 * q:32 * q + 32, :],
                                    at_sb[32 * q:32 * q + 32, :],
                                    r_ps[32 * q:32 * q + 32, :], op=OP.mult)
                            nc.vector.tensor_tensor(
                                xs[g][:], xs[g][:], at_sb[:], op=OP.add)

                    # ---- ln2 (replaces stream) -> xn2 bf16
                    xn2 = [bpool.tile([P, ROWS], BF, tag=f"xn2_{m}",
                                      name=f"xn2_{l}_{m}") for m in range(2)]
                    layernorm(l, 1, bpool, xn2, replace_stream=True)

                    # ---- FFN
                    rw1_sb = bpool.tile([P, 2048], BF, tag="rw1",
                                        name=f"rw1_{l}")
                    nc.sync.dma_start(rw1_sb[:], rw1_d[l])
                    rb1_sb = bpool.tile([P, 8], F32, tag="rb1", name=f"rb1_{l}")
                    nc.sync.dma_start(rb1_sb[:], rb1_d[l])
                    rw2_sb = bpool.tile([P, 2048], BF, tag="rw2",
                                        name=f"rw2_{l}")
                    nc.sync.dma_start(rw2_sb[:], rw2_d[l])
                    rb2_sb = bpool.tile([P, 2], F32, tag="rb2", name=f"rb2_{l}")
                    nc.sync.dma_start(rb2_sb[:], rb2_d[l])

                    rw1_v = rw1_sb.rearrange("p (m k c) -> p m k c", m=8, k=2)
                    rw2_v = rw2_sb.rearrange("p (m k c) -> p m k c", m=2, k=8)
                    with tc.tile_pool(name=f"ffps_{l}", bufs=2,
                                      space="PSUM") as fpp:
                        h1 = []
                        for mt in range(8):
                            ps = fpp.tile([P, ROWS], F32, tag="f1",
                                          name=f"f1ps_{l}_{mt}")
                            for k in range(2):
                                nc.tensor.matmul(ps[:], rw1_v[:, mt, k, :],
                                                 xn2[k][:], start=(k == 0),
                                                 stop=(k == 1))
                            o = bpool.tile([P, ROWS], BF, tag=f"h1_{mt}",
                                           name=f"h1_{l}_{mt}")
                            nc.scalar.activation(o[:], ps[:], AF.Gelu,
                                                 bias=rb1_sb[:, mt:mt + 1])
                            h1.append(o)
                        for mt in range(2):
                            ps = fpp.tile([P, ROWS], F32, tag="f2",
                                          name=f"f2ps_{l}_{mt}")
                            for k in range(8):
                                nc.tensor.matmul(ps[:], rw2_v[:, mt, k, :],
                                                 h1[k][:], start=(k == 0),
                                                 stop=(k == 7))
                            nc.vector.scalar_tensor_tensor(
                                xs[mt][:], ps[:], rb2_sb[:, mt:mt + 1],
                                xs[mt][:], op0=OP.add, op1=OP.add)

            # ---------------- output head + global standardize ----------------
            outw_sb = cpool.tile([P, 256], BF, name="outw_sb")
            nc.sync.dma_start(outw_sb[:], outw_d[:])
            outb_sb = cpool.tile([P, 1], F32, name="outb_sb")
            nc.sync.dma_start(outb_sb[:], outb_d[:])
            xfbf = [cpool.tile([P, ROWS], BF, name=f"xfbf_{m}")
                    for m in range(2)]
            for m in range(2):
                nc.vector.tensor_copy(xfbf[m][:], xs[m][:])
            with tc.tile_pool(name="fin_ps", bufs=1, space="PSUM") as opp:
                ops = opp.tile([P, ROWS], F32, name="out_ps")
                for k in range(2):
                    nc.tensor.matmul(ops[:], outw_sb[:, k * P:(k + 1) * P],
                                     xfbf[k][:], start=(k == 0), stop=(k == 1))
                out_sb = cpool.tile([P, ROWS], F32, name="out_sb")
                nc.scalar.activation(out_sb[:], ops[:], AF.Identity,
                                     bias=outb_sb[:, 0:1])
                sc = cpool.tile([P, 2], F32, name="sc")
                nc.vector.tensor_reduce(sc[:, 0:1], out_sb[:],
                                        axis=mybir.AxisListType.X, op=OP.add)
                sq_scr = cpool.tile([P, ROWS], F32, name="sq_scr")
                nc.scalar.activation(sq_scr[:], out_sb[:], AF.Square,
                                     accum_out=sc[:, 1:2])
                scbf = cpool.tile([P, 2], BF, name="scbf")
                nc.vector.tensor_copy(scbf[:], sc[:])
                tot_ps = opp.tile([1, 2], F32, name="tot_ps")
                nc.tensor.matmul(tot_ps[:], onesP_sb[:], scbf[:],
                                 start=True, stop=True)

                tot_sb = cpool.tile([1, 2], F32, name="tot_sb")
                nc.vector.tensor_copy(tot_sb[:], tot_ps[:])
                if n_cores > 1:
                    cc_in = dpool.tile([1, 2], F32, name="cc_in")
                    cc_out = dpool.tile([1, 2], F32, addr_space="Shared",
                                        name="cc_out")
                    nc.sync.dma_start(cc_in[:], tot_sb[:])
                    nc.gpsimd.collective_compute(
                        "AllReduce", OP.add,
                        replica_groups=[list(range(n_cores))],
                        ins=[cc_in[:]], outs=[cc_out[:]])
                    st_sb = cpool.tile([1, 2], F32, name="st_sb")
                    nc.sync.dma_start(st_sb[:], cc_out[:])
                else:
                    st_sb = tot_sb

                mean = cpool.tile([1, 1], F32, name="mean")
                nc.vector.tensor_scalar(mean[:], st_sb[:, 0:1], 1.0 / NTOT,
                                        None, op0=OP.mult)
                tb = cpool.tile([1, 1], F32, name="tb")
                nc.vector.tensor_tensor(tb[:], mean[:], mean[:], op=OP.mult)
                ta = cpool.tile([1, 1], F32, name="ta")
                nc.vector.tensor_scalar(ta[:], st_sb[:, 1:2],
                                        1.0 / (NTOT - 1.0), None, op0=OP.mult)
                var = cpool.tile([1, 1], F32, name="var")
                nc.vector.scalar_tensor_tensor(
                    var[:], tb[:], -NTOT / (NTOT - 1.0), ta[:],
                    op0=OP.mult, op1=OP.add)
                lnv = cpool.tile([1, 1], F32, name="lnv")
                nc.scalar.activation(lnv[:], var[:], AF.Ln)
                rs_pack = cpool.tile([1, 2], F32, name="rs_pack")
                nc.scalar.activation(rs_pack[:, 0:1], lnv[:], AF.Exp,
                                     scale=-0.5)
                tshift = cpool.tile([1, 1], F32, name="tshift")
                nc.vector.scalar_tensor_tensor(
                    tshift[:], mean[:], -1.0, rs_pack[:, 0:1],
                    op0=OP.mult, op1=OP.mult)
                nc.vector.tensor_scalar(rs_pack[:, 1:2], tshift[:], 1e-10,
                                        None, op0=OP.add)
                bc = cpool.tile([P, 2], F32, name="bc")
                nc.gpsimd.partition_broadcast(bc[:], rs_pack[:])
                nc.vector.tensor_scalar(out_sb[:], out_sb[:], bc[:, 0:1],
                                        bc[:, 1:2], op0=OP.mult, op1=OP.add)
                nc.sync.dma_start(out_d[:], out_sb[:])

    nc.compile()
    return nc


# ---------------- host-side weight prep ----------------

def _bf(a):
    return np.ascontiguousarray(a).astype(ml_dtypes.bfloat16)


def _f32(a):
    return np.ascontiguousarray(a, dtype=np.float32)


def _tile_w(w):
    """[K, M] -> [Mt, 128, Kt*128] with sb[m, p, k*128+c] = w[k*128+p, m*128+c]."""
    K, M = w.shape
    kt, mt = K // P, M // P
    return _bf(w.reshape(kt, P, mt, P).transpose(2, 1, 0, 3).reshape(mt, P, kt * P))


def _bias_grid(b):
    """[M] -> [128, Mt] with sb[p, m] = b[m*128+p]."""
    M = b.shape[0]
    return _f32(np.asarray(b).reshape(M // P, P).T)


def prep_shared(inp):
    d = {}
    for i, name in enumerate(["fc1", "fc2", "fc3", "fc4", "fc5"]):
        d[f"w{i}"] = _tile_w(np.asarray(inp[f"{name}_w"]))
        d[f"b{i}"] = _bias_grid(np.asarray(inp[f"{name}_b"]))
    d["w5"] = _tile_w(np.asarray(inp["pre_w"]))
    d["b5"] = _bias_grid(np.asarray(inp["pre_b"]))
    d["posT"] = _f32(np.asarray(inp["pos_w"])[0].T.reshape(2, P, ROWS))

    enc_w = np.asarray(inp["enc_w"])  # [NB, 256, 768]
    enc_b = np.asarray(inp["enc_b"])  # [NB, 768]
    d["encqk"] = _bf(enc_w[:, :, :512].reshape(NB, 2, P, 4, P)
                     .transpose(0, 2, 3, 1, 4).reshape(NB, P, 1024))
    d["encv"] = _bf(enc_w[:, :, 512:].reshape(NB, 2, P, 256)
                    .transpose(0, 2, 1, 3).reshape(NB, P, 512))
    d["encbqk"] = _f32(enc_b[:, :512].reshape(NB, 4, P).transpose(0, 2, 1))
    d["bvb"] = _f32(np.broadcast_to(enc_b[:, None, 512:], (NB, P, 256)))

    lngb = np.stack([
        np.concatenate([np.asarray(inp["ln1_g"]),
                        np.asarray(inp["ln1_b"])], axis=1),
        np.concatenate([np.asarray(inp["ln2_g"]),
                        np.asarray(inp["ln2_b"])], axis=1),
    ], axis=1)  # [NB, 2(which), 512 = g|b]
    d["lngb"] = _bf(lngb)

    rw1 = np.asarray(inp["res_w1"])  # [NB, 256, 1024]
    d["rw1"] = _bf(rw1.reshape(NB, 2, P, 8, P).transpose(0, 2, 3, 1, 4)
                   .reshape(NB, P, 2048))
    d["rb1"] = _f32(np.asarray(inp["res_b1"]).reshape(NB, 8, P)
                    .transpose(0, 2, 1))
    rw2 = np.asarray(inp["res_w2"])  # [NB, 1024, 256]
    d["rw2"] = _bf(rw2.reshape(NB, 8, P, 2, P).transpose(0, 2, 3, 1, 4)
                   .reshape(NB, P, 2048))
    d["rb2"] = _f32(np.asarray(inp["res_b2"]).reshape(NB, 2, P)
                    .transpose(0, 2, 1))
    ow = np.asarray(inp["out_w"])  # [256, 128]
    d["outw"] = _bf(ow.reshape(2, P, P).transpose(1, 0, 2).reshape(P, 256))
    d["outb"] = _f32(np.asarray(inp["out_b"]).reshape(P, 1))

    jj = np.arange(P)[:, None]
    ii = np.arange(P)[None, :]
    d["tri"] = _f32(np.where(ii >= jj, 0.0, NEG))
    d["m256"] = _bf(np.full((P, 1), 1.0 / 256.0))
    d["onesP"] = _bf(np.ones((P, 1)))
    d["lnones"] = _bf(np.ones((1, ROWS)))
    return d


_CACHED_NC = None
TRACE = False
LAST_RESULT = None


def kernel(**inputs) -> np.ndarray:
    global _CACHED_NC, LAST_RESULT
    if _CACHED_NC is None:
        _CACHED_NC = build_nc(8)
    nc = _CACHED_NC

    shared = prep_shared(inputs)
    state = np.asarray(inputs["state"], dtype=np.float32).reshape(B, S, 4096)
    in_maps = []
    for b in range(B):
        m = dict(shared)
        m["xT"] = _bf(state[b].T.reshape(32, P, ROWS))
        in_maps.append(m)

    res = run_bass_kernel_spmd(nc, in_maps, core_ids=list(range(8)),
                               trace=TRACE)
    LAST_RESULT = res
    out = np.stack([res.results[i]["out"] for i in range(B)])  # [B, COMP, S]
    return np.ascontiguousarray(out.transpose(0, 2, 1)).astype(np.float32)

